# revision 25
# baseline (speedup 1.0000x reference)
"""Trainium2 Bass kernel for 16-head cross-attention (B=4, T=2048, C=1024).

Sharding: tensor-parallel over heads. Each of the 8 cores computes 2 heads
end-to-end (QKV projections for its head slice, attention, and its partial
of the output projection); the host sums the 8 partials and adds bo.

Per-core dataflow (all matmuls in float32r: full PE rate, ~1e-4 precision):
  - host supplies x^T / features^T so Q^T,K^T,V^T = W^T @ x^T come out in
    [head_dim, tokens] layout directly
  - scores^T[k, q] = K^T_tile.T @ Q^T  (both heads packed via PE row groups)
  - P^T = exp(scores^T / 8) on ScalarE, reading PSUM, writing SBUF
  - V is masked (mask folded into V rows) and augmented with the mask as a
    65th column, so PV = V_aug.T @ P^T yields both the attention numerator
    and the softmax denominator (row 64) in one accumulation chain
  - normalize with reciprocal + gpsimd partition_broadcast
  - out projection accumulates both heads into one PSUM bank (K=64 each)

Cross-batch software pipelining: engine streams execute in emission order,
so stage1(b+1) and proj(b-1) instructions are interleaved into the
attention(b) loop via generators to keep ScalarE/PE busy end to end.
"""

import os

import numpy as np

os.environ.setdefault("NEURON_RT_RESET_CORES", "1")

import concourse.bass as bass
import concourse.mybir as mybir
import concourse.tile as tile
from concourse import bacc
from concourse.bass_utils import run_bass_kernel_spmd

N_CORES = 8
B = 4
T = 2048  # Tq == Tk
C = 1024
HEAD = 64
D = 128  # head dims per core (2 heads)
NT = T // 128  # 16 k/t tiles per batch
F32 = mybir.dt.float32
F32R = mybir.dt.float32r

Exp = mybir.ActivationFunctionType.Exp
Alu = mybir.AluOpType


def build_kernel():
    nc = bacc.Bacc("TRN2", target_bir_lowering=False, debug=False)

    xT = nc.dram_tensor("xT", [B, C, T], F32R, kind="ExternalInput")
    fT = nc.dram_tensor("fT", [B, C, T], F32R, kind="ExternalInput")
    maskc = nc.dram_tensor("maskc", [B, 128, NT], F32, kind="ExternalInput")
    wq = nc.dram_tensor("wq", [C, D], F32R, kind="ExternalInput")
    wk = nc.dram_tensor("wk", [C, D], F32R, kind="ExternalInput")
    wv = nc.dram_tensor("wv", [C, D], F32R, kind="ExternalInput")
    wo = nc.dram_tensor("wo", [D, C], F32R, kind="ExternalInput")
    bq = nc.dram_tensor("bq", [D, 1], F32, kind="ExternalInput")
    bk = nc.dram_tensor("bk", [D, 1], F32, kind="ExternalInput")
    bv = nc.dram_tensor("bv", [D, 1], F32, kind="ExternalInput")
    identd = nc.dram_tensor("ident", [128, 128], F32R, kind="ExternalInput")
    part = nc.dram_tensor("part", [B, T, C], F32, kind="ExternalOutput")

    NCT = C // 128  # 8 contraction tiles for the projections

    with tile.TileContext(nc) as tc:
        with (
            tc.tile_pool(name="const", bufs=1) as constp,
            tc.tile_pool(name="acts", bufs=10) as acts,
            tc.tile_pool(name="qkv", bufs=2) as qkv,
            tc.tile_pool(name="vab", bufs=2) as vab,
            tc.tile_pool(name="pbuf", bufs=5) as pbuf,
            tc.tile_pool(name="attn", bufs=1) as attnp,
            tc.tile_pool(name="outb", bufs=6) as outb,
            tc.tile_pool(name="psum", bufs=2, space="PSUM") as ps,
        ):
            # ---- constants ----
            wq_sb = constp.tile([128, NCT, D], F32R, tag="wq")
            wk_sb = constp.tile([128, NCT, D], F32R, tag="wk")
            wv_sb = constp.tile([128, NCT, D], F32R, tag="wv")
            nc.sync.dma_start(wq_sb[:], wq.ap().rearrange("(t p) m -> p t m", p=128))
            nc.sync.dma_start(wk_sb[:], wk.ap().rearrange("(t p) m -> p t m", p=128))
            nc.sync.dma_start(wv_sb[:], wv.ap().rearrange("(t p) m -> p t m", p=128))
            wo_sb = constp.tile([D, C], F32R, tag="wo")
            nc.sync.dma_start(wo_sb[:], wo.ap())
            bq_sb = constp.tile([D, 1], F32, tag="bq")
            bk_sb = constp.tile([D, 1], F32, tag="bk")
            bv_sb = constp.tile([D, 1], F32, tag="bv")
            nc.sync.dma_start(bq_sb[:], bq.ap())
            nc.sync.dma_start(bk_sb[:], bk.ap())
            nc.sync.dma_start(bv_sb[:], bv.ap())
            mask_sb = constp.tile([128, B, NT], F32, tag="mask")
            nc.sync.dma_start(mask_sb[:], maskc.ap().rearrange("b p t -> p b t"))
            ident = constp.tile([128, 128], F32R, tag="ident")
            nc.sync.dma_start(ident[:], identd.ap())

            # per-batch state handed between pipeline phases
            qT = [None] * B
            kT = [None] * B
            v_aug = [None] * B
            attn_nrm = [None] * B

            def emit_q_chunk(b, chunk):
                """One 512-column chunk of the Q projection."""
                csl = bass.ts(chunk, 512)
                ps_q = ps.tile([128, 512], F32, tag="bank1", bufs=2)
                for ct in range(NCT):
                    xt = acts.tile([128, 512], F32R, tag="xt")
                    nc.sync.dma_start(xt[:], xT.ap()[b, bass.ts(ct, 128), csl])
                    nc.tensor.matmul(
                        ps_q[:], wq_sb[:, ct, :], xt[:],
                        start=(ct == 0), stop=(ct == NCT - 1),
                    )
                    yield
                nc.vector.tensor_scalar_add(qT[b][:, csl], ps_q[:], bq_sb[:, 0:1])
                yield

            def emit_q_rest(b):
                for chunk in range(1, 4):
                    yield from emit_q_chunk(b, chunk)

            def emit_kv_chunk(b, chunk, vT):
                csl = bass.ts(chunk, 512)
                ps_k = ps.tile([128, 512], F32, tag="bank1", bufs=2)
                ps_v = ps.tile([128, 512], F32, tag="bank1", bufs=2)
                for ct in range(NCT):
                    ft = acts.tile([128, 512], F32R, tag="ft")
                    nc.sync.dma_start(ft[:], fT.ap()[b, bass.ts(ct, 128), csl])
                    nc.tensor.matmul(
                        ps_k[:], wk_sb[:, ct, :], ft[:],
                        start=(ct == 0), stop=(ct == NCT - 1),
                    )
                    nc.tensor.matmul(
                        ps_v[:], wv_sb[:, ct, :], ft[:],
                        start=(ct == 0), stop=(ct == NCT - 1),
                    )
                    yield
                nc.vector.tensor_scalar_add(kT[b][:, csl], ps_k[:], bk_sb[:, 0:1])
                nc.vector.tensor_scalar_add(vT[:, csl], ps_v[:], bv_sb[:, 0:1])
                yield

            def emit_vaug(b, g, vT):
                """V natural layout for k-tiles [4*g, 4*g+4), both heads:
                v_aug[h][k, kt, 0:64] = mask*V ; [.., 64] = mask.
                Needs only vT chunk g."""
                for h in range(2):
                    hsl = slice(h * HEAD, (h + 1) * HEAD)
                    ps_t = ps.tile([128, 4, HEAD], F32R, tag="bank1", bufs=2)
                    for j in range(4):
                        kt = g * 4 + j
                        nc.tensor.transpose(
                            ps_t[:, j, :],
                            vT[hsl, bass.ts(kt, 128)],
                            ident[hsl, hsl],
                        )
                    mrep = mask_sb[:, b, bass.ts(g, 4)].unsqueeze(2).broadcast_to(
                        [128, 4, HEAD]
                    )
                    nc.vector.tensor_tensor(
                        v_aug[b][h][:, bass.ts(g, 4), 0:HEAD],
                        ps_t[:],
                        mrep,
                        Alu.mult,
                    )
                    if g == 0:
                        nc.vector.tensor_copy(
                            v_aug[b][h][:, :, HEAD], mask_sb[:, b, :]
                        )
                    yield

            def emit_stage1(b):
                """KV + V-layout + Q, ordered so attention(b) can start after
                the 'q0done' marker: [kv c0-1, vaug(kt 0-7), q c0] suffice for
                attention(b, qc=0, kt<8); the rest streams in the background
                (vaug(kt 8-15) is emitted early enough for PV kt>=8)."""
                qT[b] = qkv.tile([128, T], F32R, tag="qT", name=f"qT_{b}")
                kT[b] = qkv.tile([128, T], F32R, tag="kT", name=f"kT_{b}")
                vT = qkv.tile([128, T], F32R, tag="vT", name=f"vT_{b}")
                v_aug[b] = [
                    vab.tile(
                        [128, NT, HEAD + 1], F32R, tag=f"vaug{h}", name=f"vaug{h}_{b}"
                    )
                    for h in range(2)
                ]
                yield from emit_kv_chunk(b, 0, vT)
                yield from emit_vaug(b, 0, vT)
                yield from emit_q_chunk(b, 0)
                yield "q0done"
                yield from emit_kv_chunk(b, 1, vT)
                yield from emit_vaug(b, 1, vT)
                yield from emit_kv_chunk(b, 2, vT)
                yield from emit_vaug(b, 2, vT)
                yield from emit_kv_chunk(b, 3, vT)
                yield from emit_vaug(b, 3, vT)
                yield from emit_q_rest(b)

            def emit_proj(b, qc):
                """Output projection for batch b, query chunk qc (4 t-tiles).
                Both heads contract in one K=128 matmul per tile."""
                for tt in range(qc * 4, qc * 4 + 4):
                    tsl = bass.ts(tt, 128)
                    for ech in range(2):
                        esl = bass.ts(ech, 512)
                        po = ps.tile([128, 512], F32, tag="bank1", bufs=2)
                        nc.tensor.matmul(
                            po[:], attn_nrm[b][:, tsl], wo_sb[:, esl],
                            start=True, stop=True,
                        )
                        o_sb = outb.tile([128, 512], F32, tag="osb")
                        nc.vector.tensor_copy(o_sb[:], po[:])
                        nc.sync.dma_start(part.ap()[b, tsl, esl], o_sb[:])
                        yield

            bgq = []  # FIFO of background generators

            def bg_step(n=1):
                for _ in range(n):
                    while bgq:
                        try:
                            next(bgq[0])
                            break
                        except StopIteration:
                            bgq.pop(0)

            def emit_attention(b):
                """Attention for batch b, draining background work between
                matmul groups to keep all engines busy."""
                attn_nrm[b] = attnp.tile(
                    [128, T], F32R, tag="attnn", bufs=2, name=f"attnn_{b}"
                )
                for qc in range(4):
                    qsl = bass.ts(qc, 512)
                    pv = [
                        ps.tile(
                            [HEAD + 1, 512], F32, tag="pv", bufs=2,
                            name=f"pv{h}_{b}_{qc}",
                        )
                        for h in range(2)
                    ]
                    def emit_pv(p_sb, kt):
                        for h in range(2):
                            nc.tensor.matmul(
                                pv[h][:],
                                v_aug[b][h][:, kt, :],
                                p_sb[:, bass.ts(h, 512)],
                                start=(kt == 0), stop=(kt == 15),
                            )

                    # kt loop pipelined by one: scores(kt) are emitted (and
                    # run on PE) before PV(kt-1), so ScalarE's exp stream
                    # never waits on next-tile scores.
                    pend = None
                    for kt in range(16):
                        s_ps = ps.tile([128, 1024], F32, tag="score")
                        p_sb = pbuf.tile([128, 1024], F32R, tag="p")
                        for h in range(2):
                            hsl = slice(h * HEAD, (h + 1) * HEAD)
                            nc.tensor.matmul(
                                s_ps[:, bass.ts(h, 512)],
                                kT[b][hsl, bass.ts(kt, 128)],
                                qT[b][hsl, qsl],
                                start=True, stop=True,
                            )
                        nc.scalar.activation(p_sb[:], s_ps[:], Exp, scale=0.125)
                        if pend is not None:
                            emit_pv(*pend)
                        bg_step(7 if (b == 0 and qc == 0) else 2)
                        pend = (p_sb, kt)
                    emit_pv(*pend)
                    # evacuate + normalize this query chunk, then queue its
                    # output projection as background work
                    for h in range(2):
                        au = attnp.tile(
                            [HEAD + 1, 512], F32, tag="au", bufs=4,
                            name=f"au{h}_{b}_{qc}",
                        )
                        nc.vector.tensor_copy(au[:], pv[h][:])
                        rrow = attnp.tile(
                            [1, 512], F32, tag="rrow", bufs=4, name=f"rr{h}_{b}_{qc}"
                        )
                        nc.vector.reciprocal(rrow[:], au[HEAD : HEAD + 1, :])
                        rbc = attnp.tile(
                            [HEAD, 512], F32, tag="rbc", bufs=4, name=f"rb{h}_{b}_{qc}"
                        )
                        nc.gpsimd.partition_broadcast(rbc[:], rrow[0:1, :])
                        nc.vector.tensor_tensor(
                            attn_nrm[b][h * HEAD : (h + 1) * HEAD, qsl],
                            au[0:HEAD, :],
                            rbc[:],
                            Alu.mult,
                        )
                    bgq.append(emit_proj(b, qc))

            # ---- pipeline ----
            g0 = emit_stage1(0)
            for step in g0:
                if step == "q0done":
                    break
            bgq.append(g0)
            for b in range(B):
                if b + 1 < B:
                    bgq.append(emit_stage1(b + 1))
                emit_attention(b)
            while bgq:
                bg_step(1)

    nc.compile()
    return nc


_NC = None
last_in_maps = None


def kernel(x, features, mask, wq, bq, wk, bk, wv, bv, wo, bo):
    global _NC
    global last_in_maps
    x = np.asarray(x, dtype=np.float32)
    features = np.asarray(features, dtype=np.float32)
    mask_np = np.asarray(mask)
    wq = np.asarray(wq, dtype=np.float32)
    wk = np.asarray(wk, dtype=np.float32)
    wv = np.asarray(wv, dtype=np.float32)
    wo = np.asarray(wo, dtype=np.float32)
    bq = np.asarray(bq, dtype=np.float32)
    bk = np.asarray(bk, dtype=np.float32)
    bv = np.asarray(bv, dtype=np.float32)
    bo = np.asarray(bo, dtype=np.float32)

    xT = np.ascontiguousarray(x.transpose(0, 2, 1))
    fT = np.ascontiguousarray(features.transpose(0, 2, 1))
    # mask [B,1,1,T] -> [B, 128, NT] with maskc[b, p, t] = mask[b, t*128+p]
    maskc = np.ascontiguousarray(
        mask_np.reshape(B, NT, 128).transpose(0, 2, 1).astype(np.float32)
    )

    in_maps = []
    for c in range(N_CORES):
        dsl = slice(c * D, (c + 1) * D)
        in_maps.append(
            {
                "xT": xT,
                "fT": fT,
                "maskc": maskc,
                "wq": np.ascontiguousarray(wq[:, dsl]),
                "wk": np.ascontiguousarray(wk[:, dsl]),
                "wv": np.ascontiguousarray(wv[:, dsl]),
                "wo": np.ascontiguousarray(wo[dsl, :]),
                "bq": np.ascontiguousarray(bq[dsl]).reshape(D, 1),
                "bk": np.ascontiguousarray(bk[dsl]).reshape(D, 1),
                "bv": np.ascontiguousarray(bv[dsl]).reshape(D, 1),
                "ident": np.eye(128, dtype=np.float32),
            }
        )

    last_in_maps = in_maps
    if _NC is None:
        _NC = build_kernel()

    res = None
    last_exc = None
    for _attempt in range(3):
        try:
            res = run_bass_kernel_spmd(_NC, in_maps, core_ids=list(range(N_CORES)))
            break
        except Exception as e:  # intermittent device-init faults: retry
            last_exc = e
            import time as _time

            _time.sleep(2.0)
    if res is None:
        raise last_exc
    out = res.results[0]["part"].astype(np.float64)
    for c in range(1, N_CORES):
        out += res.results[c]["part"].astype(np.float64)
    out += bo.astype(np.float64)
    return out.astype(np.float32)


if __name__ == "__main__":
    rng = np.random.default_rng(0)
    ins = {
        "x": rng.standard_normal((B, T, C)).astype(np.float32),
        "features": rng.standard_normal((B, T, C)).astype(np.float32),
        "mask": (rng.integers(0, 2, (B, 1, 1, T))).astype(np.int32),
        "wq": (rng.standard_normal((C, C)) / 32).astype(np.float32),
        "bq": np.zeros(C, np.float32),
        "wk": (rng.standard_normal((C, C)) / 32).astype(np.float32),
        "bk": np.zeros(C, np.float32),
        "wv": (rng.standard_normal((C, C)) / 32).astype(np.float32),
        "bv": np.zeros(C, np.float32),
        "wo": (rng.standard_normal((C, C)) / 32).astype(np.float32),
        "bo": np.zeros(C, np.float32),
    }
    out = kernel(**ins)
    print("kernel output", out.shape, out.dtype, float(np.abs(out).mean()))


# revision 26
# speedup vs baseline: 1.0020x; 1.0020x over previous
"""Trainium2 Bass kernel for 16-head cross-attention (B=4, T=2048, C=1024).

Sharding: tensor-parallel over heads. Each of the 8 cores computes 2 heads
end-to-end (QKV projections for its head slice, attention, and its partial
of the output projection); the host sums the 8 partials and adds bo.

Per-core dataflow (all matmuls in float32r: full PE rate, ~1e-4 precision):
  - host supplies x^T / features^T so Q^T,K^T,V^T = W^T @ x^T come out in
    [head_dim, tokens] layout directly
  - scores^T[k, q] = K^T_tile.T @ Q^T  (both heads packed via PE row groups)
  - P^T = exp(scores^T / 8) on ScalarE, reading PSUM, writing SBUF
  - V is masked (mask folded into V rows) and augmented with the mask as a
    65th column, so PV = V_aug.T @ P^T yields both the attention numerator
    and the softmax denominator (row 64) in one accumulation chain
  - normalize with reciprocal + gpsimd partition_broadcast
  - out projection accumulates both heads into one PSUM bank (K=64 each)

Cross-batch software pipelining: engine streams execute in emission order,
so stage1(b+1) and proj(b-1) instructions are interleaved into the
attention(b) loop via generators to keep ScalarE/PE busy end to end.
"""

import os

import numpy as np

os.environ.setdefault("NEURON_RT_RESET_CORES", "1")

import concourse.bass as bass
import concourse.mybir as mybir
import concourse.tile as tile
from concourse import bacc
from concourse.bass_utils import run_bass_kernel_spmd

N_CORES = 8
B = 4
T = 2048  # Tq == Tk
C = 1024
HEAD = 64
D = 128  # head dims per core (2 heads)
NT = T // 128  # 16 k/t tiles per batch
F32 = mybir.dt.float32
F32R = mybir.dt.float32r

Exp = mybir.ActivationFunctionType.Exp
Alu = mybir.AluOpType


def build_kernel():
    nc = bacc.Bacc("TRN2", target_bir_lowering=False, debug=False)

    xT = nc.dram_tensor("xT", [B, C, T], F32R, kind="ExternalInput")
    fT = nc.dram_tensor("fT", [B, C, T], F32R, kind="ExternalInput")
    maskc = nc.dram_tensor("maskc", [B, 128, NT], F32, kind="ExternalInput")
    wq = nc.dram_tensor("wq", [C, D], F32R, kind="ExternalInput")
    wk = nc.dram_tensor("wk", [C, D], F32R, kind="ExternalInput")
    wv = nc.dram_tensor("wv", [C, D], F32R, kind="ExternalInput")
    wo = nc.dram_tensor("wo", [D, C], F32R, kind="ExternalInput")
    bq = nc.dram_tensor("bq", [D, 1], F32, kind="ExternalInput")
    bk = nc.dram_tensor("bk", [D, 1], F32, kind="ExternalInput")
    bv = nc.dram_tensor("bv", [D, 1], F32, kind="ExternalInput")
    identd = nc.dram_tensor("ident", [128, 128], F32R, kind="ExternalInput")
    part = nc.dram_tensor("part", [B, T, C], F32, kind="ExternalOutput")

    NCT = C // 128  # 8 contraction tiles for the projections

    with tile.TileContext(nc) as tc:
        with (
            tc.tile_pool(name="const", bufs=1) as constp,
            tc.tile_pool(name="acts", bufs=10) as acts,
            tc.tile_pool(name="qkv", bufs=2) as qkv,
            tc.tile_pool(name="vab", bufs=2) as vab,
            tc.tile_pool(name="pbuf", bufs=5) as pbuf,
            tc.tile_pool(name="attn", bufs=1) as attnp,
            tc.tile_pool(name="outb", bufs=6) as outb,
            tc.tile_pool(name="psum", bufs=2, space="PSUM") as ps,
        ):
            # ---- constants ----
            wq_sb = constp.tile([128, NCT, D], F32R, tag="wq")
            wk_sb = constp.tile([128, NCT, D], F32R, tag="wk")
            wv_sb = constp.tile([128, NCT, D], F32R, tag="wv")
            nc.sync.dma_start(wq_sb[:], wq.ap().rearrange("(t p) m -> p t m", p=128))
            nc.sync.dma_start(wk_sb[:], wk.ap().rearrange("(t p) m -> p t m", p=128))
            nc.sync.dma_start(wv_sb[:], wv.ap().rearrange("(t p) m -> p t m", p=128))
            wo_sb = constp.tile([D, C], F32R, tag="wo")
            nc.sync.dma_start(wo_sb[:], wo.ap())
            bq_sb = constp.tile([D, 1], F32, tag="bq")
            bk_sb = constp.tile([D, 1], F32, tag="bk")
            bv_sb = constp.tile([D, 1], F32, tag="bv")
            nc.sync.dma_start(bq_sb[:], bq.ap())
            nc.sync.dma_start(bk_sb[:], bk.ap())
            nc.sync.dma_start(bv_sb[:], bv.ap())
            mask_sb = constp.tile([128, B, NT], F32, tag="mask")
            nc.sync.dma_start(mask_sb[:], maskc.ap().rearrange("b p t -> p b t"))
            ident = constp.tile([128, 128], F32R, tag="ident")
            nc.sync.dma_start(ident[:], identd.ap())

            # per-batch state handed between pipeline phases
            qT = [None] * B
            kT = [None] * B
            v_aug = [None] * B
            attn_nrm = [None] * B

            def emit_q_chunk(b, chunk):
                """One 512-column chunk of the Q projection."""
                csl = bass.ts(chunk, 512)
                ps_q = ps.tile([128, 512], F32, tag="bank1", bufs=2)
                for ct in range(NCT):
                    xt = acts.tile([128, 512], F32R, tag="xt")
                    nc.sync.dma_start(xt[:], xT.ap()[b, bass.ts(ct, 128), csl])
                    nc.tensor.matmul(
                        ps_q[:], wq_sb[:, ct, :], xt[:],
                        start=(ct == 0), stop=(ct == NCT - 1),
                    )
                    yield
                nc.vector.tensor_scalar_add(qT[b][:, csl], ps_q[:], bq_sb[:, 0:1])
                yield

            def emit_q_rest(b):
                for chunk in range(1, 4):
                    yield from emit_q_chunk(b, chunk)

            def emit_kv_chunk(b, chunk, vT):
                csl = bass.ts(chunk, 512)
                ps_k = ps.tile([128, 512], F32, tag="bank1", bufs=2)
                ps_v = ps.tile([128, 512], F32, tag="bank1", bufs=2)
                for ct in range(NCT):
                    ft = acts.tile([128, 512], F32R, tag="ft")
                    nc.sync.dma_start(ft[:], fT.ap()[b, bass.ts(ct, 128), csl])
                    nc.tensor.matmul(
                        ps_k[:], wk_sb[:, ct, :], ft[:],
                        start=(ct == 0), stop=(ct == NCT - 1),
                    )
                    nc.tensor.matmul(
                        ps_v[:], wv_sb[:, ct, :], ft[:],
                        start=(ct == 0), stop=(ct == NCT - 1),
                    )
                    yield
                nc.vector.tensor_scalar_add(kT[b][:, csl], ps_k[:], bk_sb[:, 0:1])
                nc.vector.tensor_scalar_add(vT[:, csl], ps_v[:], bv_sb[:, 0:1])
                yield

            def emit_vaug(b, g, vT):
                """V natural layout for k-tiles [4*g, 4*g+4), both heads:
                v_aug[h][k, kt, 0:64] = mask*V ; [.., 64] = mask.
                Needs only vT chunk g."""
                for h in range(2):
                    hsl = slice(h * HEAD, (h + 1) * HEAD)
                    ps_t = ps.tile([128, 4, HEAD], F32R, tag="bank1", bufs=2)
                    for j in range(4):
                        kt = g * 4 + j
                        nc.tensor.transpose(
                            ps_t[:, j, :],
                            vT[hsl, bass.ts(kt, 128)],
                            ident[hsl, hsl],
                        )
                    mrep = mask_sb[:, b, bass.ts(g, 4)].unsqueeze(2).broadcast_to(
                        [128, 4, HEAD]
                    )
                    nc.vector.tensor_tensor(
                        v_aug[b][h][:, bass.ts(g, 4), 0:HEAD],
                        ps_t[:],
                        mrep,
                        Alu.mult,
                    )
                    if g == 0:
                        nc.vector.tensor_copy(
                            v_aug[b][h][:, :, HEAD], mask_sb[:, b, :]
                        )
                    yield

            def emit_stage1(b):
                """KV + V-layout + Q, ordered so attention(b) can start after
                the 'q0done' marker: [kv c0-1, vaug(kt 0-7), q c0] suffice for
                attention(b, qc=0, kt<8); the rest streams in the background
                (vaug(kt 8-15) is emitted early enough for PV kt>=8)."""
                qT[b] = qkv.tile([128, T], F32R, tag="qT", name=f"qT_{b}")
                kT[b] = qkv.tile([128, T], F32R, tag="kT", name=f"kT_{b}")
                vT = qkv.tile([128, T], F32R, tag="vT", name=f"vT_{b}")
                v_aug[b] = [
                    vab.tile(
                        [128, NT, HEAD + 1], F32R, tag=f"vaug{h}", name=f"vaug{h}_{b}"
                    )
                    for h in range(2)
                ]
                yield from emit_kv_chunk(b, 0, vT)
                yield from emit_vaug(b, 0, vT)
                yield from emit_q_chunk(b, 0)
                yield "q0done"
                yield from emit_kv_chunk(b, 1, vT)
                yield from emit_vaug(b, 1, vT)
                yield from emit_kv_chunk(b, 2, vT)
                yield from emit_vaug(b, 2, vT)
                yield from emit_kv_chunk(b, 3, vT)
                yield from emit_vaug(b, 3, vT)
                yield from emit_q_rest(b)

            def emit_proj(b, qc):
                """Output projection for batch b, query chunk qc (4 t-tiles).
                Both heads contract in one K=128 matmul per tile."""
                for tt in range(qc * 4, qc * 4 + 4):
                    tsl = bass.ts(tt, 128)
                    for ech in range(2):
                        esl = bass.ts(ech, 512)
                        po = ps.tile([128, 512], F32, tag="bank1", bufs=2)
                        nc.tensor.matmul(
                            po[:], attn_nrm[b][:, tsl], wo_sb[:, esl],
                            start=True, stop=True,
                        )
                        o_sb = outb.tile([128, 512], F32, tag="osb")
                        nc.vector.tensor_copy(o_sb[:], po[:])
                        nc.sync.dma_start(part.ap()[b, tsl, esl], o_sb[:])
                        yield

            bgq = []  # FIFO of background generators

            def bg_step(n=1):
                for _ in range(n):
                    while bgq:
                        try:
                            next(bgq[0])
                            break
                        except StopIteration:
                            bgq.pop(0)

            def emit_attention(b):
                """Attention for batch b, draining background work between
                matmul groups to keep all engines busy."""
                attn_nrm[b] = attnp.tile(
                    [128, T], F32R, tag="attnn", bufs=2, name=f"attnn_{b}"
                )
                for qc in range(4):
                    qsl = bass.ts(qc, 512)
                    pv = [
                        ps.tile(
                            [HEAD + 1, 512], F32, tag="pv", bufs=2,
                            name=f"pv{h}_{b}_{qc}",
                        )
                        for h in range(2)
                    ]
                    def emit_pv(p_sb, kt):
                        for h in range(2):
                            nc.tensor.matmul(
                                pv[h][:],
                                v_aug[b][h][:, kt, :],
                                p_sb[:, bass.ts(h, 512)],
                                start=(kt == 0), stop=(kt == 15),
                            )

                    # kt loop pipelined by one: scores(kt) are emitted (and
                    # run on PE) before PV(kt-1), so ScalarE's exp stream
                    # never waits on next-tile scores.
                    pend = None
                    for kt in range(16):
                        s_ps = ps.tile([128, 1024], F32, tag="score")
                        p_sb = pbuf.tile([128, 1024], F32R, tag="p")
                        for h in range(2):
                            hsl = slice(h * HEAD, (h + 1) * HEAD)
                            nc.tensor.matmul(
                                s_ps[:, bass.ts(h, 512)],
                                kT[b][hsl, bass.ts(kt, 128)],
                                qT[b][hsl, qsl],
                                start=True, stop=True,
                            )
                        nc.scalar.activation(p_sb[:], s_ps[:], Exp, scale=0.125)
                        if pend is not None:
                            emit_pv(*pend)
                        if b == 0 and qc == 0:
                            bg_step(7)
                        else:
                            bg_step(0 if kt == 0 else (4 if kt == 15 else 2))
                        pend = (p_sb, kt)
                    emit_pv(*pend)
                    # evacuate + normalize this query chunk, then queue its
                    # output projection as background work
                    for h in range(2):
                        au = attnp.tile(
                            [HEAD + 1, 512], F32, tag="au", bufs=6,
                            name=f"au{h}_{b}_{qc}",
                        )
                        nc.vector.tensor_copy(au[:], pv[h][:])
                        rrow = attnp.tile(
                            [1, 512], F32, tag="rrow", bufs=6, name=f"rr{h}_{b}_{qc}"
                        )
                        nc.vector.reciprocal(rrow[:], au[HEAD : HEAD + 1, :])
                        rbc = attnp.tile(
                            [HEAD, 512], F32, tag="rbc", bufs=6, name=f"rb{h}_{b}_{qc}"
                        )
                        nc.gpsimd.partition_broadcast(rbc[:], rrow[0:1, :])
                        nc.vector.tensor_tensor(
                            attn_nrm[b][h * HEAD : (h + 1) * HEAD, qsl],
                            au[0:HEAD, :],
                            rbc[:],
                            Alu.mult,
                        )
                    bgq.append(emit_proj(b, qc))

            # ---- pipeline ----
            g0 = emit_stage1(0)
            for step in g0:
                if step == "q0done":
                    break
            bgq.append(g0)
            for b in range(B):
                if b + 1 < B:
                    bgq.append(emit_stage1(b + 1))
                emit_attention(b)
            while bgq:
                bg_step(1)

    nc.compile()
    return nc


_NC = None
last_in_maps = None


def kernel(x, features, mask, wq, bq, wk, bk, wv, bv, wo, bo):
    global _NC
    global last_in_maps
    x = np.asarray(x, dtype=np.float32)
    features = np.asarray(features, dtype=np.float32)
    mask_np = np.asarray(mask)
    wq = np.asarray(wq, dtype=np.float32)
    wk = np.asarray(wk, dtype=np.float32)
    wv = np.asarray(wv, dtype=np.float32)
    wo = np.asarray(wo, dtype=np.float32)
    bq = np.asarray(bq, dtype=np.float32)
    bk = np.asarray(bk, dtype=np.float32)
    bv = np.asarray(bv, dtype=np.float32)
    bo = np.asarray(bo, dtype=np.float32)

    xT = np.ascontiguousarray(x.transpose(0, 2, 1))
    fT = np.ascontiguousarray(features.transpose(0, 2, 1))
    # mask [B,1,1,T] -> [B, 128, NT] with maskc[b, p, t] = mask[b, t*128+p]
    maskc = np.ascontiguousarray(
        mask_np.reshape(B, NT, 128).transpose(0, 2, 1).astype(np.float32)
    )

    in_maps = []
    for c in range(N_CORES):
        dsl = slice(c * D, (c + 1) * D)
        in_maps.append(
            {
                "xT": xT,
                "fT": fT,
                "maskc": maskc,
                "wq": np.ascontiguousarray(wq[:, dsl]),
                "wk": np.ascontiguousarray(wk[:, dsl]),
                "wv": np.ascontiguousarray(wv[:, dsl]),
                "wo": np.ascontiguousarray(wo[dsl, :]),
                "bq": np.ascontiguousarray(bq[dsl]).reshape(D, 1),
                "bk": np.ascontiguousarray(bk[dsl]).reshape(D, 1),
                "bv": np.ascontiguousarray(bv[dsl]).reshape(D, 1),
                "ident": np.eye(128, dtype=np.float32),
            }
        )

    last_in_maps = in_maps
    if _NC is None:
        _NC = build_kernel()

    res = None
    last_exc = None
    for _attempt in range(3):
        try:
            res = run_bass_kernel_spmd(_NC, in_maps, core_ids=list(range(N_CORES)))
            break
        except Exception as e:  # intermittent device-init faults: retry
            last_exc = e
            import time as _time

            _time.sleep(2.0)
    if res is None:
        raise last_exc
    out = res.results[0]["part"].astype(np.float64)
    for c in range(1, N_CORES):
        out += res.results[c]["part"].astype(np.float64)
    out += bo.astype(np.float64)
    return out.astype(np.float32)


if __name__ == "__main__":
    rng = np.random.default_rng(0)
    ins = {
        "x": rng.standard_normal((B, T, C)).astype(np.float32),
        "features": rng.standard_normal((B, T, C)).astype(np.float32),
        "mask": (rng.integers(0, 2, (B, 1, 1, T))).astype(np.int32),
        "wq": (rng.standard_normal((C, C)) / 32).astype(np.float32),
        "bq": np.zeros(C, np.float32),
        "wk": (rng.standard_normal((C, C)) / 32).astype(np.float32),
        "bk": np.zeros(C, np.float32),
        "wv": (rng.standard_normal((C, C)) / 32).astype(np.float32),
        "bv": np.zeros(C, np.float32),
        "wo": (rng.standard_normal((C, C)) / 32).astype(np.float32),
        "bo": np.zeros(C, np.float32),
    }
    out = kernel(**ins)
    print("kernel output", out.shape, out.dtype, float(np.abs(out).mean()))
